# revision 7
# baseline (speedup 1.0000x reference)
"""Trainium2 Bass kernel for the E2V hypergraph message-passing layer.

Reference computation:
    edge_i = hyperedge[ve_affiliation[0]]          # [N_INC, 64]
    edge_j = hyperedge[ve_affiliation[1]]          # [N_INC, 64]
    x = concat(edge_i, edge_j, hyper_node)         # [N_INC, 192]
    out = relu(x @ W.T + b)                        # [N_INC, 64]

Strategy: data-parallel over the incidence dimension across 8 cores.
The host performs the index expansion (pure data movement: gathering
edge rows per incidence and laying them out feature-major); the device
streams all tensors once and performs the full 192->64 fused linear +
ReLU (all model FLOPs on device). Layouts are chosen so every DMA
touches all 128 SBUF partitions (full port bandwidth):

  eij_t [128, S]    partitions 0-63 = edge_i features, 64-127 = edge_j
                    features of the same incidence column. The edge term
                    is then ONE K=128 matmul with lhsT = [Wi.T ; Wj.T].
  node2 [128, S/2]  incidence halves stacked on partition halves:
                    node2[0:64, c]   = hyper_node.T[:, c]
                    node2[64:128, c] = hyper_node.T[:, c + S/2]
  out2  [128, S/2]  same half-stacking; host un-stacks + transposes.

Per 512-column output slice (= 1024 incidences), one PSUM bank [128,512]
holds both halves; 4 matmuls accumulate (edge K=128 + node K=64 per
half, hi-half via tile_position quadrants), then ScalarE applies
bias+ReLU at full 128-lane width.
"""

import numpy as np

import concourse.tile as tile
from concourse import bacc, mybir
from concourse.bass_utils import run_bass_kernel_spmd

# Problem constants (hardcoded; kernel.py must be self-contained).
N_EDGES = 100000
N_INC = 2000000
D = 64
N_CORES = 8

BLK = 2048          # out2 columns per block (= 4096 incidences)
SUB = 512           # PSUM free-dim per accumulation group


def _derived(shard):
    nblk = -(-shard // (2 * BLK))          # blocks over the half domain
    return nblk, nblk * 2 * BLK            # (NBLK, SHARD_PAD)


NBLK, SHARD_PAD = _derived(N_INC // N_CORES)   # 62, 253952


def build(nc, nblk=NBLK):
    f32 = mybir.dt.float32
    s = nblk * 2 * BLK
    half = s // 2

    eij_t = nc.dram_tensor("eij_t", [128, s], f32, kind="ExternalInput")
    node2 = nc.dram_tensor("node2", [128, half], f32, kind="ExternalInput")
    w_ij = nc.dram_tensor("w_ij", [128, D], f32, kind="ExternalInput")
    wn2 = nc.dram_tensor("wn2", [128, D], f32, kind="ExternalInput")
    bias2 = nc.dram_tensor("bias2", [128, 1], f32, kind="ExternalInput")
    out2 = nc.dram_tensor("out2", [128, half], f32, kind="ExternalOutput")

    with tile.TileContext(nc) as tc:
        with (
            tc.tile_pool(name="const", bufs=1) as const_pool,
            tc.tile_pool(name="work", bufs=3) as work_pool,
            tc.tile_pool(name="psum", bufs=6, space="PSUM") as psum_pool,
        ):
            wij_sb = const_pool.tile([128, D], f32)
            nc.sync.dma_start(wij_sb[:], w_ij[:])
            wn2_sb = const_pool.tile([128, D], f32)
            nc.sync.dma_start(wn2_sb[:], wn2[:])
            bia = const_pool.tile([128, 1], f32)
            nc.sync.dma_start(bia[:], bias2[:])

            for k in range(nblk):
                c0 = k * BLK
                eia = work_pool.tile([128, BLK], f32, tag="eia")
                nc.sync.dma_start(eia[:], eij_t[:, c0:c0 + BLK])
                eib = work_pool.tile([128, BLK], f32, tag="eib")
                nc.sync.dma_start(eib[:], eij_t[:, half + c0:half + c0 + BLK])
                ntile = work_pool.tile([128, BLK], f32, tag="ntile")
                nc.sync.dma_start(ntile[:], node2[:, c0:c0 + BLK])
                otile = work_pool.tile([128, BLK], f32, tag="otile")
                for si in range(BLK // SUB):
                    sl = slice(si * SUB, (si + 1) * SUB)
                    ps = psum_pool.tile([128, SUB], f32, tag="ps")
                    # low half: incidences c0+sl
                    nc.tensor.matmul(
                        ps[0:D, :], lhsT=wij_sb[:], rhs=eia[:, sl],
                        start=True, stop=False,
                    )
                    nc.tensor.matmul(
                        ps[0:D, :], lhsT=wn2_sb[0:D, :], rhs=ntile[0:D, sl],
                        start=False, stop=True,
                    )
                    # high half: incidences half+c0+sl
                    nc.tensor.matmul(
                        ps[D:128, :], lhsT=wij_sb[:], rhs=eib[:, sl],
                        start=True, stop=False, tile_position=(0, 64),
                    )
                    nc.tensor.matmul(
                        ps[D:128, :], lhsT=wn2_sb[D:128, :], rhs=ntile[D:128, sl],
                        start=False, stop=True, tile_position=(64, 64),
                    )
                    nc.scalar.activation(
                        out=otile[:, sl], in_=ps[:],
                        func=mybir.ActivationFunctionType.Relu, bias=bia[:],
                    )
                nc.sync.dma_start(out2[:, c0:c0 + BLK], otile[:])
    return nc


def make_host_inputs(hyperedge, hyper_node, ve_affiliation, W, b,
                     n_cores=N_CORES, nblk=NBLK):
    """Shard + index-expand + lay out full inputs into per-core in_maps."""
    s = nblk * 2 * BLK
    half = s // 2
    n_inc = hyper_node.shape[0]
    shard = n_inc // n_cores

    hyperedge = np.asarray(hyperedge, dtype=np.float32)
    hyper_node = np.asarray(hyper_node, dtype=np.float32)
    ve = np.asarray(ve_affiliation)
    W = np.asarray(W, dtype=np.float32)
    b = np.asarray(b, dtype=np.float32)

    # lhsT for the K=128 edge matmul: rows 0-63 = Wi.T, 64-127 = Wj.T
    w_ij = np.ascontiguousarray(np.concatenate([W[:, :D].T, W[:, D:2 * D].T], axis=0))
    wn2 = np.ascontiguousarray(np.concatenate([W[:, 2 * D:].T, W[:, 2 * D:].T], axis=0))
    bias2 = np.concatenate([b, b]).reshape(128, 1).copy()

    in_maps = []
    for c in range(n_cores):
        sl = slice(c * shard, (c + 1) * shard)
        eij = np.zeros((128, s), dtype=np.float32)
        eij[0:D, :shard] = hyperedge[ve[0, sl]].T
        eij[D:128, :shard] = hyperedge[ve[1, sl]].T
        nT = np.zeros((D, s), dtype=np.float32)
        nT[:, :shard] = hyper_node[sl].T
        node2 = np.concatenate([nT[:, :half], nT[:, half:]], axis=0)
        in_maps.append(dict(
            eij_t=eij,
            node2=np.ascontiguousarray(node2),
            w_ij=w_ij,
            wn2=wn2,
            bias2=bias2,
        ))
    return in_maps


_CACHE = {}


def _get_nc():
    if "nc" not in _CACHE:
        nc = bacc.Bacc("TRN2", target_bir_lowering=False, debug=False)
        build(nc)
        nc.finalize()  # runs bacc passes incl. register allocation
        _CACHE["nc"] = nc
    return _CACHE["nc"]


def kernel(hyperedge, hyper_node, ve_affiliation, W, b, _spmd_kwargs=None):
    n_inc = np.asarray(hyper_node).shape[0]
    shard = n_inc // N_CORES
    half = SHARD_PAD // 2
    in_maps = make_host_inputs(hyperedge, hyper_node, ve_affiliation, W, b)
    nc = _get_nc()
    res = run_bass_kernel_spmd(
        nc, in_maps, core_ids=list(range(N_CORES)), **(_spmd_kwargs or {})
    )
    outs = []
    for r in res.results:
        o2 = r["out2"]  # [128, half]
        ot = np.concatenate([o2[0:D, :], o2[D:128, :]], axis=1)  # [64, S]
        outs.append(ot[:, :shard].T)
    out = np.ascontiguousarray(np.concatenate(outs, axis=0), dtype=np.float32)
    if _spmd_kwargs:
        return out, res
    return out


# revision 12
# speedup vs baseline: 1.6253x; 1.6253x over previous
"""Trainium2 Bass kernel for the E2V hypergraph message-passing layer.

Reference computation:
    edge_i = hyperedge[ve_affiliation[0]]          # [N_INC, 64]
    edge_j = hyperedge[ve_affiliation[1]]          # [N_INC, 64]
    x = concat(edge_i, edge_j, hyper_node)         # [N_INC, 192]
    out = relu(x @ W.T + b)                        # [N_INC, 64]

Strategy: data-parallel over the incidence dimension across 8 cores.
The host performs the index expansion (pure data movement: gathering
edge rows per incidence and laying them out feature-major); the device
streams all tensors once and performs the full 192->64 fused linear +
ReLU (all model FLOPs on device). Layouts are chosen so every DMA
touches all 128 SBUF partitions (full port bandwidth):

  eij_t [128, S]    partitions 0-63 = edge_i features, 64-127 = edge_j
                    features of the same incidence column. The edge term
                    is then ONE K=128 matmul with lhsT = [Wi.T ; Wj.T].
  node2 [128, S/2]  incidence halves stacked on partition halves:
                    node2[0:64, c]   = hyper_node.T[:, c]
                    node2[64:128, c] = hyper_node.T[:, c + S/2]
  out2  [128, S/2]  same half-stacking; host un-stacks + transposes.

Per 512-column output slice (= 1024 incidences), one PSUM bank [128,512]
holds both halves; 4 matmuls accumulate (edge K=128 + node K=64 per
half, hi-half via tile_position quadrants), then ScalarE applies
bias+ReLU at full 128-lane width.
"""

import ml_dtypes
import numpy as np

import concourse.tile as tile
from concourse import bacc, mybir
from concourse.bass_utils import run_bass_kernel_spmd

# Problem constants (hardcoded; kernel.py must be self-contained).
N_EDGES = 100000
N_INC = 2000000
D = 64
N_CORES = 8

BLK = 2048          # out2 columns per block (= 4096 incidences)
SUB = 512           # PSUM free-dim per accumulation group


def _derived(shard):
    nblk = -(-shard // (2 * BLK))          # blocks over the half domain
    return nblk, nblk * 2 * BLK            # (NBLK, SHARD_PAD)


NBLK, SHARD_PAD = _derived(N_INC // N_CORES)   # 62, 253952


def build(nc, nblk=NBLK):
    f32 = mybir.dt.float32
    bf16 = mybir.dt.bfloat16
    s = nblk * 2 * BLK
    half = s // 2

    # Inputs bf16 (PE runs bf16 at 2x fp32 rate + FWL; PSUM accumulates
    # f32, output stays f32), halving input DMA traffic as well.
    eij_t = nc.dram_tensor("eij_t", [128, s], bf16, kind="ExternalInput")
    node2 = nc.dram_tensor("node2", [128, half], bf16, kind="ExternalInput")
    w_ij = nc.dram_tensor("w_ij", [128, D], bf16, kind="ExternalInput")
    wn2 = nc.dram_tensor("wn2", [128, D], bf16, kind="ExternalInput")
    bias2 = nc.dram_tensor("bias2", [128, 1], f32, kind="ExternalInput")
    out2 = nc.dram_tensor("out2", [128, half], f32, kind="ExternalOutput")

    with tile.TileContext(nc) as tc:
        with (
            tc.tile_pool(name="const", bufs=1) as const_pool,
            tc.tile_pool(name="work", bufs=3) as work_pool,
            tc.tile_pool(name="psum", bufs=6, space="PSUM") as psum_pool,
        ):
            wij_sb = const_pool.tile([128, D], bf16)
            nc.sync.dma_start(wij_sb[:], w_ij[:])
            wn2_sb = const_pool.tile([128, D], bf16)
            nc.sync.dma_start(wn2_sb[:], wn2[:])
            bia = const_pool.tile([128, 1], f32)
            nc.sync.dma_start(bia[:], bias2[:])

            for k in range(nblk):
                c0 = k * BLK
                eia = work_pool.tile([128, BLK], bf16, tag="eia")
                nc.sync.dma_start(eia[:], eij_t[:, c0:c0 + BLK])
                eib = work_pool.tile([128, BLK], bf16, tag="eib")
                nc.sync.dma_start(eib[:], eij_t[:, half + c0:half + c0 + BLK])
                ntile = work_pool.tile([128, BLK], bf16, tag="ntile")
                nc.sync.dma_start(ntile[:], node2[:, c0:c0 + BLK])
                otile = work_pool.tile([128, BLK], f32, tag="otile")
                for si in range(BLK // SUB):
                    sl = slice(si * SUB, (si + 1) * SUB)
                    ps = psum_pool.tile([128, SUB], f32, tag="ps")
                    # low half: incidences c0+sl
                    nc.tensor.matmul(
                        ps[0:D, :], lhsT=wij_sb[:], rhs=eia[:, sl],
                        start=True, stop=False,
                    )
                    nc.tensor.matmul(
                        ps[0:D, :], lhsT=wn2_sb[0:D, :], rhs=ntile[0:D, sl],
                        start=False, stop=True,
                    )
                    # high half: incidences half+c0+sl
                    nc.tensor.matmul(
                        ps[D:128, :], lhsT=wij_sb[:], rhs=eib[:, sl],
                        start=True, stop=False, tile_position=(0, 64),
                    )
                    nc.tensor.matmul(
                        ps[D:128, :], lhsT=wn2_sb[D:128, :], rhs=ntile[D:128, sl],
                        start=False, stop=True, tile_position=(64, 64),
                    )
                    nc.scalar.activation(
                        out=otile[:, sl], in_=ps[:],
                        func=mybir.ActivationFunctionType.Relu, bias=bia[:],
                    )
                nc.sync.dma_start(out2[:, c0:c0 + BLK], otile[:])
    return nc


def make_host_inputs(hyperedge, hyper_node, ve_affiliation, W, b,
                     n_cores=N_CORES, nblk=NBLK):
    """Shard + index-expand + lay out full inputs into per-core in_maps."""
    s = nblk * 2 * BLK
    half = s // 2
    n_inc = hyper_node.shape[0]
    shard = n_inc // n_cores

    hyperedge = np.asarray(hyperedge, dtype=np.float32)
    hyper_node = np.asarray(hyper_node, dtype=np.float32)
    ve = np.asarray(ve_affiliation)
    W = np.asarray(W, dtype=np.float32)
    b = np.asarray(b, dtype=np.float32)

    bf = ml_dtypes.bfloat16
    # lhsT for the K=128 edge matmul: rows 0-63 = Wi.T, 64-127 = Wj.T
    w_ij = np.ascontiguousarray(
        np.concatenate([W[:, :D].T, W[:, D:2 * D].T], axis=0).astype(bf))
    wn2 = np.ascontiguousarray(
        np.concatenate([W[:, 2 * D:].T, W[:, 2 * D:].T], axis=0).astype(bf))
    bias2 = np.concatenate([b, b]).reshape(128, 1).astype(np.float32)

    hyperedge_bf_t = np.ascontiguousarray(hyperedge.astype(bf).T)  # [64, E]

    in_maps = []
    for c in range(n_cores):
        sl = slice(c * shard, (c + 1) * shard)
        eij = np.zeros((128, s), dtype=bf)
        eij[0:D, :shard] = hyperedge_bf_t[:, ve[0, sl]]
        eij[D:128, :shard] = hyperedge_bf_t[:, ve[1, sl]]
        nT = np.zeros((D, s), dtype=bf)
        nT[:, :shard] = hyper_node[sl].astype(bf).T
        node2 = np.concatenate([nT[:, :half], nT[:, half:]], axis=0)
        in_maps.append(dict(
            eij_t=eij,
            node2=np.ascontiguousarray(node2),
            w_ij=w_ij,
            wn2=wn2,
            bias2=bias2,
        ))
    return in_maps


_CACHE = {}


def _get_nc():
    if "nc" not in _CACHE:
        nc = bacc.Bacc("TRN2", target_bir_lowering=False, debug=False)
        build(nc)
        nc.finalize()  # runs bacc passes incl. register allocation
        _CACHE["nc"] = nc
    return _CACHE["nc"]


def kernel(hyperedge, hyper_node, ve_affiliation, W, b, _spmd_kwargs=None):
    n_inc = np.asarray(hyper_node).shape[0]
    shard = n_inc // N_CORES
    half = SHARD_PAD // 2
    in_maps = make_host_inputs(hyperedge, hyper_node, ve_affiliation, W, b)
    nc = _get_nc()
    res = run_bass_kernel_spmd(
        nc, in_maps, core_ids=list(range(N_CORES)), **(_spmd_kwargs or {})
    )
    outs = []
    for r in res.results:
        o2 = r["out2"]  # [128, half]
        ot = np.concatenate([o2[0:D, :], o2[D:128, :]], axis=1)  # [64, S]
        outs.append(ot[:, :shard].T)
    out = np.ascontiguousarray(np.concatenate(outs, axis=0), dtype=np.float32)
    if _spmd_kwargs:
        return out, res
    return out


# revision 17
# speedup vs baseline: 2.5323x; 1.5581x over previous
"""Trainium2 Bass kernel for the E2V hypergraph message-passing layer.

Reference computation:
    edge_i = hyperedge[ve_affiliation[0]]          # [N_INC, 64]
    edge_j = hyperedge[ve_affiliation[1]]          # [N_INC, 64]
    x = concat(edge_i, edge_j, hyper_node)         # [N_INC, 192]
    out = relu(x @ W.T + b)                        # [N_INC, 64]

Strategy: data-parallel over the incidence dimension across 8 cores.
The host performs the index expansion (pure data movement: gathering
edge rows per incidence and laying them out feature-major); the device
streams all tensors once and performs the full 192->64 fused linear +
ReLU (all model FLOPs on device). bf16 operands with f32 PSUM
accumulation (PE 2x rate + half DMA traffic); output stored bf16 and
upconverted on host. Layouts keep every DMA on all 128 SBUF partitions:

  eij_t [128, S]    block-interleaved: for block k, cols [2k*B,(2k+1)*B)
                    hold the LOW incidence half's edge features
                    (partitions 0-63 = edge_i, 64-127 = edge_j), cols
                    [(2k+1)*B,(2k+2)*B) the HIGH half's. One K=128
                    matmul per half with lhsT = [Wi.T ; Wj.T].
  node2 [128, S/2]  incidence halves stacked on partition halves.
  out2  [128, S/2]  same half-stacking; host un-stacks + transposes.

Per 512-column output slice (= 1024 incidences), one PSUM bank
[128,512]: a single K=128 block-diagonal [[Wn.T,0],[0,Wn.T]] matmul
computes BOTH node halves (start=True), then the two edge matmuls
accumulate (hi half via tile_position col-group 64), then ScalarE
applies bias+ReLU at full 128-lane width.
"""

import ml_dtypes
import numpy as np

import concourse.tile as tile
from concourse import bacc, mybir
from concourse.bass_utils import run_bass_kernel_spmd

# Problem constants (hardcoded; kernel.py must be self-contained).
N_EDGES = 100000
N_INC = 2000000
D = 64
N_CORES = 8

BLK = 4096          # out2 columns per block (= 8192 incidences)
SUB = 512           # PSUM free-dim per accumulation group


def _derived(shard):
    nblk = -(-shard // (2 * BLK))          # blocks over the half domain
    return nblk, nblk * 2 * BLK            # (NBLK, SHARD_PAD)


NBLK, SHARD_PAD = _derived(N_INC // N_CORES)   # 31, 253952


def build(nc, nblk=NBLK):
    f32 = mybir.dt.float32
    bf16 = mybir.dt.bfloat16
    s = nblk * 2 * BLK
    half = s // 2

    eij_t = nc.dram_tensor("eij_t", [128, s], bf16, kind="ExternalInput")
    node2 = nc.dram_tensor("node2", [128, half], bf16, kind="ExternalInput")
    w_ij = nc.dram_tensor("w_ij", [128, D], bf16, kind="ExternalInput")
    wn_bd = nc.dram_tensor("wn_bd", [128, 128], bf16, kind="ExternalInput")
    bias2 = nc.dram_tensor("bias2", [128, 1], f32, kind="ExternalInput")
    out2 = nc.dram_tensor("out2", [128, half], bf16, kind="ExternalOutput")

    with tile.TileContext(nc) as tc:
        with (
            tc.tile_pool(name="const", bufs=1) as const_pool,
            tc.tile_pool(name="work", bufs=4) as work_pool,
            tc.tile_pool(name="psum", bufs=8, space="PSUM") as psum_pool,
        ):
            wij_sb = const_pool.tile([128, D], bf16)
            nc.sync.dma_start(wij_sb[:], w_ij[:])
            wnbd_sb = const_pool.tile([128, 128], bf16)
            nc.sync.dma_start(wnbd_sb[:], wn_bd[:])
            bia = const_pool.tile([128, 1], f32)
            nc.sync.dma_start(bia[:], bias2[:])

            for k in range(nblk):
                c0 = k * BLK
                # one contiguous 2MB load covers both incidence halves
                epair = work_pool.tile([128, 2 * BLK], bf16, tag="epair")
                nc.sync.dma_start(epair[:], eij_t[:, 2 * c0:2 * c0 + 2 * BLK])
                ntile = work_pool.tile([128, BLK], bf16, tag="ntile")
                nc.sync.dma_start(ntile[:], node2[:, c0:c0 + BLK])
                otile = work_pool.tile([128, BLK], bf16, tag="otile")
                for si in range(BLK // SUB):
                    sl = slice(si * SUB, (si + 1) * SUB)
                    ps = psum_pool.tile([128, SUB], f32, tag="ps")
                    # both node halves in one block-diagonal K=128 matmul
                    nc.tensor.matmul(
                        ps[:], lhsT=wnbd_sb[:], rhs=ntile[:, sl],
                        start=True, stop=False, skip_group_check=True,
                    )
                    # edge halves accumulate; both share the wij stationary
                    nc.tensor.matmul(
                        ps[0:D, :], lhsT=wij_sb[:], rhs=epair[:, sl],
                        start=False, stop=True, skip_group_check=True,
                    )
                    nc.tensor.matmul(
                        ps[D:128, :],
                        lhsT=wij_sb[:],
                        rhs=epair[:, BLK + si * SUB:BLK + (si + 1) * SUB],
                        start=False, stop=True, skip_group_check=True,
                        tile_position=(0, 64),
                    )
                    nc.scalar.activation(
                        out=otile[:, sl], in_=ps[:],
                        func=mybir.ActivationFunctionType.Relu, bias=bia[:],
                    )
                nc.sync.dma_start(out2[:, c0:c0 + BLK], otile[:])
    return nc


def make_host_inputs(hyperedge, hyper_node, ve_affiliation, W, b,
                     n_cores=N_CORES, nblk=NBLK):
    """Shard + index-expand + lay out full inputs into per-core in_maps."""
    s = nblk * 2 * BLK
    half = s // 2
    n_inc = hyper_node.shape[0]
    shard = n_inc // n_cores

    hyperedge = np.asarray(hyperedge, dtype=np.float32)
    hyper_node = np.asarray(hyper_node, dtype=np.float32)
    ve = np.asarray(ve_affiliation)
    W = np.asarray(W, dtype=np.float32)
    b = np.asarray(b, dtype=np.float32)

    bf = ml_dtypes.bfloat16
    # lhsT for the K=128 edge matmul: rows 0-63 = Wi.T, 64-127 = Wj.T
    w_ij = np.ascontiguousarray(
        np.concatenate([W[:, :D].T, W[:, D:2 * D].T], axis=0).astype(bf))
    wn_bd = np.zeros((128, 128), dtype=bf)
    wn_bd[0:D, 0:D] = W[:, 2 * D:].T.astype(bf)
    wn_bd[D:128, D:128] = W[:, 2 * D:].T.astype(bf)
    bias2 = np.concatenate([b, b]).reshape(128, 1).astype(np.float32)

    hyperedge_bf_t = np.ascontiguousarray(hyperedge.astype(bf).T)  # [64, E]

    in_maps = []
    for c in range(n_cores):
        sl = slice(c * shard, (c + 1) * shard)
        eij = np.zeros((128, s), dtype=bf)
        eij[0:D, :shard] = hyperedge_bf_t[:, ve[0, sl]]
        eij[D:128, :shard] = hyperedge_bf_t[:, ve[1, sl]]
        # interleave so block k's lo/hi halves are adjacent (one DMA)
        lo = eij[:, :half].reshape(128, nblk, BLK)
        hi = eij[:, half:].reshape(128, nblk, BLK)
        eij_il = np.stack([lo, hi], axis=2).reshape(128, s)
        nT = np.zeros((D, s), dtype=bf)
        nT[:, :shard] = hyper_node[sl].astype(bf).T
        node2 = np.concatenate([nT[:, :half], nT[:, half:]], axis=0)
        in_maps.append(dict(
            eij_t=np.ascontiguousarray(eij_il),
            node2=np.ascontiguousarray(node2),
            w_ij=w_ij,
            wn_bd=wn_bd,
            bias2=bias2,
        ))
    return in_maps


_CACHE = {}


def _get_nc():
    if "nc" not in _CACHE:
        nc = bacc.Bacc("TRN2", target_bir_lowering=False, debug=False)
        build(nc)
        nc.finalize()  # runs bacc passes incl. register allocation
        _CACHE["nc"] = nc
    return _CACHE["nc"]


def kernel(hyperedge, hyper_node, ve_affiliation, W, b, _spmd_kwargs=None):
    n_inc = np.asarray(hyper_node).shape[0]
    shard = n_inc // N_CORES
    in_maps = make_host_inputs(hyperedge, hyper_node, ve_affiliation, W, b)
    nc = _get_nc()
    res = run_bass_kernel_spmd(
        nc, in_maps, core_ids=list(range(N_CORES)), **(_spmd_kwargs or {})
    )
    outs = []
    for r in res.results:
        o2 = r["out2"].astype(np.float32)  # [128, half]
        ot = np.concatenate([o2[0:D, :], o2[D:128, :]], axis=1)  # [64, S]
        outs.append(ot[:, :shard].T)
    out = np.ascontiguousarray(np.concatenate(outs, axis=0), dtype=np.float32)
    if _spmd_kwargs:
        return out, res
    return out
